# revision 5
# baseline (speedup 1.0000x reference)
"""Trainium2 Bass kernel for nn_Char2Token2Mention (gather + segment-sum).

    ft = token_ft[token_code]               # [NNZ, D] gather
    weighted = ft * spm_vals[:, None]
    out = segment_sum(weighted, spm_rows, num_segments=N_MENTIONS)

Strategy (8-core SPMD, mentions sharded):
  - core i owns mentions [i*8192, (i+1)*8192); spm_rows is sorted so its nnz
    form a contiguous slice. 64 windows of 128 mentions per core.
  - host casts token_ft to bf16 (tolerance 2e-2; bf16 lands ~3e-3) -- halves
    gather traffic and runs the PE at bf16 rate (fp32 PSUM accumulate).
  - gather via gpsimd.dma_gather (ant extended-ISA): ONE instruction gathers
    G*M*128 rows, amortizing the ~1us SWDGE fixed cost ~100x vs per-chunk
    indirect DMAs (the old bottleneck: 576 x 1.16us of Q7 descriptor
    generation). dma_gather indices are int16, so the 262144-row table is
    addressed as 8 segments of 32768 rows (in_ap slice supplies the segment
    base); the host sorts each window's nnz by segment and pads each
    (window, segment) group to M*128 slots with dummy valid codes (val=0).
  - device, per group of G windows: 8 dma_gathers (one per segment) fill a
    [128, 8*G*M, 256] bf16 tile; row j of a gather lands at partition j%128,
    column j//128 (snake). Per window: 8*M matmuls sel.T @ ft accumulate the
    [128, 256] f32 output in PSUM, where DVE builds
    sel[j, m] = (iota==row_j)*val_j from host-prepared row/val tables.
  - per window: PSUM -> SBUF copy -> DMA to the core's [8192, 256] output.
  - host concatenates the 8 core outputs.
"""
import os
import numpy as np
import ml_dtypes

import concourse.bacc as bacc
import concourse.bass as bass
import concourse.mybir as mybir
import concourse.tile as tile
from concourse.bass_utils import run_bass_kernel_spmd

P = 128
D = 256
N_TOKENS = 262144
NNZ = 524288
N_MENTIONS = 65536
N_CORES = 8
MENT_PER_CORE = N_MENTIONS // N_CORES          # 8192
WIN_PER_CORE = MENT_PER_CORE // P              # 64
N_WINDOWS = N_MENTIONS // P                    # 512
N_SEG = 8
SEG_ROWS = N_TOKENS // N_SEG                   # 32768 (int16-addressable)

# Results of the last run (set by kernel()); test.py reads exec_time_ns.
LAST_RESULTS = None

_nc_cache = {}


def _build_nc(G: int, M: int) -> bass.Bass:
    """G windows per gather group, M 128-slot chunks per (window, segment)."""
    n_groups = WIN_PER_CORE // G
    gcols = N_SEG * G * M                      # ft columns per group tile
    n_chunks = WIN_PER_CORE * N_SEG * M        # total sel/matmul chunks
    nper = G * M * P                           # idxs per dma_gather
    icols = nper // 16                         # idx free dim per gather
    bf16 = mybir.dt.bfloat16
    nc = bacc.Bacc("TRN2", target_bir_lowering=False, debug=False)
    table = nc.declare_dram_parameter(
        "token_ft", [N_TOKENS, D], bf16, isOutput=False
    )
    idxs = nc.declare_dram_parameter(
        "idxs", [P, n_groups * N_SEG * icols], mybir.dt.int16, isOutput=False
    )
    rows = nc.declare_dram_parameter(
        "rows", [P, n_chunks], mybir.dt.float32, isOutput=False
    )
    vals = nc.declare_dram_parameter(
        "vals", [P, n_chunks], mybir.dt.float32, isOutput=False
    )
    iota = nc.declare_dram_parameter(
        "iota", [P, P], mybir.dt.float32, isOutput=False
    )
    out = nc.declare_dram_parameter(
        "out", [MENT_PER_CORE, D], mybir.dt.float32, isOutput=True
    )

    ft_bufs = 2 if gcols * D * 2 >= 48 * 1024 else 3

    with tile.TileContext(nc) as tc:
        with (
            tc.tile_pool(name="const", bufs=1) as const_pool,
            tc.tile_pool(name="ftp", bufs=ft_bufs) as ft_pool,
            tc.tile_pool(name="selp", bufs=12) as sel_pool,
            tc.tile_pool(name="psum", bufs=4, space="PSUM") as psum_pool,
            tc.tile_pool(name="outp", bufs=4) as out_pool,
        ):
            idx_sb = const_pool.tile(
                [P, n_groups * N_SEG * icols], mybir.dt.int16
            )
            rows_sb = const_pool.tile([P, n_chunks], mybir.dt.float32)
            vals_sb = const_pool.tile([P, n_chunks], mybir.dt.float32)
            iota_sb = const_pool.tile([P, P], mybir.dt.float32)
            nc.sync.dma_start(out=idx_sb[:], in_=idxs[:])
            nc.sync.dma_start(out=rows_sb[:], in_=rows[:])
            nc.sync.dma_start(out=vals_sb[:], in_=vals[:])
            nc.sync.dma_start(out=iota_sb[:], in_=iota[:])

            for g in range(n_groups):
                ftg = ft_pool.tile([P, gcols, D], bf16, tag="ft")
                for s in range(N_SEG):
                    icol0 = (g * N_SEG + s) * icols
                    nc.gpsimd.dma_gather(
                        out_ap=ftg[:, s * G * M : (s + 1) * G * M, :],
                        in_ap=table[s * SEG_ROWS : (s + 1) * SEG_ROWS, :],
                        idxs_ap=idx_sb[:, icol0 : icol0 + icols],
                        num_idxs=nper,
                        num_idxs_reg=nper,
                        elem_size=D,
                    )
                for w_sub in range(G):
                    w = g * G + w_sub
                    psum = psum_pool.tile(
                        [P, D], mybir.dt.float32, space="PSUM", tag="acc"
                    )
                    nch = N_SEG * M
                    for i in range(nch):
                        s, c2 = i // M, i % M
                        c = s * (G * M) + w_sub * M + c2
                        gc = g * (N_SEG * G * M) + c
                        sel = sel_pool.tile([P, P], bf16, tag="sel")
                        nc.vector.tensor_scalar(
                            out=sel[:],
                            in0=iota_sb[:],
                            scalar1=rows_sb[:, gc : gc + 1],
                            scalar2=vals_sb[:, gc : gc + 1],
                            op0=mybir.AluOpType.is_equal,
                            op1=mybir.AluOpType.mult,
                        )
                        nc.tensor.matmul(
                            out=psum[:],
                            lhsT=sel[:],
                            rhs=ftg[:, c, :],
                            start=(i == 0),
                            stop=(i == nch - 1),
                        )
                    outt = out_pool.tile([P, D], mybir.dt.float32, tag="out")
                    nc.vector.tensor_copy(out=outt[:], in_=psum[:])
                    nc.sync.dma_start(
                        out=out[w * P : (w + 1) * P, :], in_=outt[:]
                    )
    nc.compile()
    return nc


def kernel(token_ft, token_code, spm_rows, spm_vals):
    global LAST_RESULTS
    ft = np.ascontiguousarray(
        np.asarray(token_ft, dtype=np.float32).astype(ml_dtypes.bfloat16)
    )
    codes = np.asarray(token_code).astype(np.int64, copy=False)
    rows = np.asarray(spm_rows).astype(np.int64, copy=False)
    vals = np.asarray(spm_vals, dtype=np.float32)
    if not np.all(rows[:-1] <= rows[1:]):
        order = np.argsort(rows, kind="stable")
        rows, codes, vals = rows[order], codes[order], vals[order]

    # sort nnz by (window, segment); windows of P mentions, segments of
    # SEG_ROWS token codes
    win = rows // P
    seg = codes // SEG_ROWS
    order = np.lexsort((seg, win))
    codes, rows, vals, win, seg = (
        codes[order], rows[order], vals[order], win[order], seg[order]
    )

    # counts per (window, segment)
    ws = win * N_SEG + seg
    cnt = np.bincount(ws, minlength=N_WINDOWS * N_SEG).reshape(
        N_WINDOWS, N_SEG
    )
    M = max(1, int(np.ceil(cnt.max() / P)))
    # G windows per gather group: <=1024 idxs per dma_gather (2048 crashes
    # the Q7 ucode) and group ft tile <= ~64KB/partition
    G = 8
    while G > 1 and (
        G * M * P > 1024 or N_SEG * G * M * D * 2 > 64 * 1024
    ):
        G //= 2
    slots_ws = M * P

    # slot index for each nnz inside its (window, segment) group
    ws_start = np.zeros(N_WINDOWS * N_SEG, np.int64)
    ws_start[1:] = np.cumsum(cnt.ravel())[:-1]
    pos = np.arange(NNZ, dtype=np.int64) - ws_start[ws]
    slot = ws * slots_ws + pos                 # global padded slot id

    n_slots = N_WINDOWS * N_SEG * slots_ws
    idx_flat = np.zeros(n_slots, np.int16)     # pad: local code 0 (valid)
    rows_flat = np.zeros(n_slots, np.float32)  # pad: mention 0, val 0
    vals_flat = np.zeros(n_slots, np.float32)
    idx_flat[slot] = (codes - seg * SEG_ROWS).astype(np.int16)
    rows_flat[slot] = (rows - win * P).astype(np.float32)
    vals_flat[slot] = vals

    iota = np.ascontiguousarray(
        np.broadcast_to(np.arange(P, dtype=np.float32), (P, P))
    )

    n_groups = WIN_PER_CORE // G
    nper = G * M * P
    icols = nper // 16
    n_chunks = WIN_PER_CORE * N_SEG * M

    in_maps = []
    for i in range(N_CORES):
        # this core's slots: [window, segment, M*P] with window-major layout
        lo = i * WIN_PER_CORE * N_SEG * slots_ws
        hi = (i + 1) * WIN_PER_CORE * N_SEG * slots_ws
        idx_c = idx_flat[lo:hi].reshape(WIN_PER_CORE, N_SEG, slots_ws)
        rows_c = rows_flat[lo:hi].reshape(WIN_PER_CORE, N_SEG, slots_ws)
        vals_c = vals_flat[lo:hi].reshape(WIN_PER_CORE, N_SEG, slots_ws)

        # gather pass (g, s) = concat over w_sub of idx_c[g*G+w_sub, s]
        # -> [n_groups, N_SEG, G*M*P]; wrap each pass [16, icols], replicate
        idx_p = idx_c.reshape(n_groups, G, N_SEG, slots_ws).transpose(
            0, 2, 1, 3
        ).reshape(n_groups * N_SEG, nper)
        idx_w = idx_p.reshape(-1, icols, 16).transpose(0, 2, 1)  # [gp,16,icols]
        idx_up = np.tile(idx_w, (1, 8, 1)).transpose(1, 0, 2).reshape(
            P, n_groups * N_SEG * icols
        )

        # sel tables per chunk: group-tile column c = s*(G*M)+w_sub*M+c2,
        # global chunk gc = g*(N_SEG*G*M) + c; its slots are
        # (w=g*G+w_sub, s, c2*P + p) for p in 0..127
        rv = rows_c.reshape(n_groups, G, N_SEG, M, P).transpose(
            0, 2, 1, 3, 4
        ).reshape(-1, P)                        # [n_chunks, P] chunk-major
        vv = vals_c.reshape(n_groups, G, N_SEG, M, P).transpose(
            0, 2, 1, 3, 4
        ).reshape(-1, P)
        in_maps.append(
            {
                "token_ft": ft,
                "idxs": np.ascontiguousarray(idx_up),
                "rows": np.ascontiguousarray(rv.T),
                "vals": np.ascontiguousarray(vv.T),
                "iota": iota,
            }
        )

    if (G, M) not in _nc_cache:
        _nc_cache[(G, M)] = _build_nc(G, M)
    nc = _nc_cache[(G, M)]

    trace = bool(os.environ.get("BASS_KERNEL_TRACE"))
    LAST_RESULTS = run_bass_kernel_spmd(
        nc, in_maps, list(range(N_CORES)), trace=trace
    )
    return np.concatenate(
        [LAST_RESULTS.results[i]["out"] for i in range(N_CORES)], axis=0
    )


# revision 6
# speedup vs baseline: 2.7516x; 2.7516x over previous
"""Trainium2 Bass kernel for nn_Char2Token2Mention (gather + segment-sum).

    ft = token_ft[token_code]               # [NNZ, D] gather
    weighted = ft * spm_vals[:, None]
    out = segment_sum(weighted, spm_rows, num_segments=N_MENTIONS)

Strategy (8-core SPMD, mentions sharded):
  - core i owns mentions [i*8192, (i+1)*8192); spm_rows is sorted so its nnz
    form a contiguous slice.
  - host bins nnz into 128-mention windows (64 per core), pads each window to
    cpw chunks of 128 nnz (pad entries: code 0 with val 0, so they fetch a
    real row that the sel matrix zeroes out -- no bounds check needed).
  - host casts token_ft to bf16 (tolerance is 2e-2; bf16 lands ~2e-3) --
    halves gather DMA traffic and runs the PE at bf16 rate (f32 PSUM).
  - device, per chunk: indirect-DMA gather of 128 rows [128, 256] bf16 from
    the (replicated) token_ft table; DVE builds sel[j, m] = (iota==row_j)
    *val_j; PE matmul sel.T @ ft accumulates the window's [128, 256] f32
    output in PSUM.
  - per window: PSUM -> SBUF copy -> DMA to the core's [8192, 256] output.
  - host concatenates the 8 core outputs.
"""
import os
import numpy as np
import ml_dtypes

import concourse.bacc as bacc
import concourse.bass as bass
import concourse.mybir as mybir
import concourse.tile as tile
from concourse.bass_utils import run_bass_kernel_spmd

P = 128
D = 256
N_TOKENS = 262144
NNZ = 524288
N_MENTIONS = 65536
N_CORES = 8
MENT_PER_CORE = N_MENTIONS // N_CORES          # 8192
WIN_PER_CORE = MENT_PER_CORE // P              # 64
N_WINDOWS = N_MENTIONS // P                    # 512
MIN_CPW = 9

# Results of the last run (set by kernel()); test.py reads exec_time_ns.
LAST_RESULTS = None

_nc_cache = {}


def _build_nc(cpw: int) -> bass.Bass:
    n_chunks = WIN_PER_CORE * cpw
    bf16 = mybir.dt.bfloat16
    nc = bacc.Bacc("TRN2", target_bir_lowering=False, debug=False)
    table = nc.declare_dram_parameter(
        "token_ft", [N_TOKENS, D], bf16, isOutput=False
    )
    codes = nc.declare_dram_parameter(
        "codes", [P, n_chunks], mybir.dt.int32, isOutput=False
    )
    rows = nc.declare_dram_parameter(
        "rows", [P, n_chunks], mybir.dt.float32, isOutput=False
    )
    vals = nc.declare_dram_parameter(
        "vals", [P, n_chunks], mybir.dt.float32, isOutput=False
    )
    iota = nc.declare_dram_parameter(
        "iota", [P, P], mybir.dt.float32, isOutput=False
    )
    out = nc.declare_dram_parameter(
        "out", [MENT_PER_CORE, D], mybir.dt.float32, isOutput=True
    )

    with tile.TileContext(nc) as tc:
        with (
            tc.tile_pool(name="const", bufs=1) as const_pool,
            tc.tile_pool(name="work", bufs=24) as work_pool,
            tc.tile_pool(name="psum", bufs=4, space="PSUM") as psum_pool,
            tc.tile_pool(name="outp", bufs=4) as out_pool,
        ):
            codes_sb = const_pool.tile([P, n_chunks], mybir.dt.int32)
            rows_sb = const_pool.tile([P, n_chunks], mybir.dt.float32)
            vals_sb = const_pool.tile([P, n_chunks], mybir.dt.float32)
            iota_sb = const_pool.tile([P, P], mybir.dt.float32)
            nc.sync.dma_start(out=codes_sb[:], in_=codes[:])
            nc.sync.dma_start(out=rows_sb[:], in_=rows[:])
            nc.sync.dma_start(out=vals_sb[:], in_=vals[:])
            nc.sync.dma_start(out=iota_sb[:], in_=iota[:])

            for w in range(WIN_PER_CORE):
                psum = psum_pool.tile(
                    [P, D], mybir.dt.float32, space="PSUM", tag="acc"
                )
                for c in range(cpw):
                    k = w * cpw + c
                    ft = work_pool.tile([P, D], bf16, tag="ft")
                    nc.gpsimd.indirect_dma_start(
                        out=ft[:],
                        out_offset=None,
                        in_=table[:],
                        in_offset=bass.IndirectOffsetOnAxis(
                            ap=codes_sb[:, k : k + 1], axis=0
                        ),
                    )
                    sel = work_pool.tile([P, P], bf16, tag="sel")
                    nc.vector.tensor_scalar(
                        out=sel[:],
                        in0=iota_sb[:],
                        scalar1=rows_sb[:, k : k + 1],
                        scalar2=vals_sb[:, k : k + 1],
                        op0=mybir.AluOpType.is_equal,
                        op1=mybir.AluOpType.mult,
                    )
                    nc.tensor.matmul(
                        out=psum[:],
                        lhsT=sel[:],
                        rhs=ft[:],
                        start=(c == 0),
                        stop=(c == cpw - 1),
                    )
                outt = out_pool.tile([P, D], mybir.dt.float32, tag="out")
                nc.vector.tensor_copy(out=outt[:], in_=psum[:])
                nc.sync.dma_start(out=out[w * P : (w + 1) * P, :], in_=outt[:])
    nc.compile()
    return nc


def kernel(token_ft, token_code, spm_rows, spm_vals):
    global LAST_RESULTS
    ft = np.ascontiguousarray(
        np.asarray(token_ft, dtype=np.float32).astype(ml_dtypes.bfloat16)
    )
    codes = np.asarray(token_code).astype(np.int64, copy=False)
    rows = np.asarray(spm_rows).astype(np.int64, copy=False)
    vals = np.asarray(spm_vals, dtype=np.float32)
    if not np.all(rows[:-1] <= rows[1:]):
        order = np.argsort(rows, kind="stable")
        rows, codes, vals = rows[order], codes[order], vals[order]

    # bin nnz into 128-mention windows (rows is sorted)
    wb = np.searchsorted(rows, np.arange(0, N_MENTIONS + 1, P)).astype(np.int64)
    counts = np.diff(wb)
    cpw = max(MIN_CPW, int(np.ceil(counts.max() / P)))
    padn = cpw * P

    wid = np.repeat(np.arange(N_WINDOWS, dtype=np.int64), counts)
    pos = np.arange(NNZ, dtype=np.int64) - np.repeat(wb[:-1], counts)

    # pads: code 0 (valid fetch), mention 0, val 0 -> zeroed by sel
    codes_p = np.zeros((N_WINDOWS, padn), np.int32)
    rows_p = np.zeros((N_WINDOWS, padn), np.float32)
    vals_p = np.zeros((N_WINDOWS, padn), np.float32)
    codes_p[wid, pos] = codes.astype(np.int32)
    rows_p[wid, pos] = (rows - wid * P).astype(np.float32)
    vals_p[wid, pos] = vals

    iota = np.ascontiguousarray(
        np.broadcast_to(np.arange(P, dtype=np.float32), (P, P))
    )

    in_maps = []
    for i in range(N_CORES):
        sl = slice(i * WIN_PER_CORE, (i + 1) * WIN_PER_CORE)
        in_maps.append(
            {
                "token_ft": ft,
                "codes": np.ascontiguousarray(
                    codes_p[sl].reshape(-1, P).T
                ),
                "rows": np.ascontiguousarray(rows_p[sl].reshape(-1, P).T),
                "vals": np.ascontiguousarray(vals_p[sl].reshape(-1, P).T),
                "iota": iota,
            }
        )

    if cpw not in _nc_cache:
        _nc_cache[cpw] = _build_nc(cpw)
    nc = _nc_cache[cpw]

    trace = bool(os.environ.get("BASS_KERNEL_TRACE"))
    LAST_RESULTS = run_bass_kernel_spmd(
        nc, in_maps, list(range(N_CORES)), trace=trace
    )
    return np.concatenate(
        [LAST_RESULTS.results[i]["out"] for i in range(N_CORES)], axis=0
    )
